# revision 44
# baseline (speedup 1.0000x reference)
"""Trainium2 Bass kernel for nn_ChebySemi_70222715289681.

out = x + (f - conv3x3(x, kernelA)) / 6   (per-sample 3x3 kernels,
B=64 images of 512x512, fp32). Pure data parallel: batch sharded 8
samples per core across 8 NeuronCores, zero communication.

Per-core kernel (batch-transposed striped layout, bf16 wire format,
106.7us baseline -> ~56us):
  Host ships x TRANSPOSED+row-padded to [H+2, B, W] bf16 and
  g = x + f/6 as [H, B, W] bf16, so one image row across all 8 samples
  is 8KB contiguous in HBM: every stripe DMA moves ~1MB in 8KB
  per-partition descriptors (row-per-partition layouts with 1-2KB
  descriptors measured only ~77-147 GB/s/queue vs ~190-220 here, and
  any partition range not starting at 0 degenerates to ONE SDMA
  engine at ~27 GB/s - hence the host-side zero halo rows).
  The image is processed in 5 row-stripes (4 x 126 output rows + an
  8-row tail); a stripe tile [128, 8*512] holds rows 126s-1..126s+126
  one-row-per-partition. With adjacent rows on adjacent partitions the
  conv's three ROW taps collapse into one banded stationary matrix
  W_dj[p,c] = -k[p-c,dj]/6, so each sample needs only 3 matmuls per
  stripe - the column taps dj ride on shifted PSUM output windows
  (dj=1 full width with start=True, dj=0 into cols 1.., dj=2 into
  cols ..511), which also kills all column padding. x itself never
  passes through the PE: the host fold g = x + f/6 makes the single
  fused DVE blend out = g + psum complete the update. Matmuls issue
  dj-major across samples into 4 two-bank PSUM pair tiles (ILP across
  banks; one wide [126, 1024] blend per pair halves DVE overhead),
  and 16 zero matmuls warm the PE HAM clock gate (starts at ~half
  clock; K=128 activity opens it) while the first loads fly.
  The 24 banded weights are built host-side from kernelA and shipped
  as one [128, 24, 126] bf16 tensor, loaded first on the Scalar ring.
  Loads issue on Sync (x) / Scalar (wts, g) HWDGE rings, stores on
  the GpSimd SWDGE ring so a store waiting on compute never
  head-of-line-blocks a load. Output is stored bf16 [H, B, W]; host
  casts/untransposes to f32 [B,1,H,W]. All wire tensors are bf16
  (13.5MB/core total vs 25.2 in f32; rel err ~2.5e-3, gate 2e-2).
"""
import numpy as np
import concourse.bass as bass
import concourse.mybir as mybir
from concourse.tile import TileContext
from concourse.bass_utils import run_bass_kernel_spmd

F32 = mybir.dt.float32
BF16 = mybir.dt.bfloat16
NPBF16 = mybir.dt.np(BF16)
ALU = mybir.AluOpType

N_CORES = 8
BPC = 8          # samples per core
H = W = 512
SH = 126         # output rows per full stripe
NS = 5           # stripes (4 full + tail)
TAIL = H - 4 * SH  # 8

_MAX_WAITS = 1


def _fixup_sync_waits(nc):
    """This walrus build rejects >1-2 sem-waits per instruction; move the
    excess onto NOPs inserted just before, on the same engine (same program
    order, so semantics are unchanged)."""
    n_fix = 0
    for fn in nc.m.functions:
        for blk in fn.blocks:
            out, changed = [], False
            for inst in blk.instructions:
                si = inst.sync_info
                waits = list(si.on_wait or []) if si is not None else []
                if len(waits) > _MAX_WAITS:
                    changed = True
                    n_fix += 1
                    for i in range(0, len(waits) - _MAX_WAITS, _MAX_WAITS):
                        nop = mybir.InstNoOp(
                            name=f"I-waitfix-{nc.next_id()}", ins=[], outs=[])
                        nop.engine = inst.engine
                        nop.sync_info = mybir.SyncInfo(
                            on_wait=waits[i:i + _MAX_WAITS], on_update=[])
                        out.append(nop)
                    inst.sync_info = mybir.SyncInfo(
                        on_wait=waits[len(waits) - _MAX_WAITS:],
                        on_update=list(si.on_update or []))
                out.append(inst)
            if changed:
                blk.instructions = out
    return n_fix


def gen_kernel(n_samples=BPC):
    nc = bass.Bass(target_bir_lowering=False)
    # x is host-padded with a zero row on top and bottom ([H+2, B, W]) so
    # every stripe load covers a partition range starting at 0: a dst
    # partition range starting elsewhere (e.g. [1:128]) defeats the
    # DGE's per-engine descriptor split - all descriptors land on ONE
    # SDMA engine and the transfer serializes at ~27 GB/s.
    x = nc.dram_tensor("x", [H + 2, n_samples, W], BF16,
                       kind="ExternalInput")
    g = nc.dram_tensor("g", [H, n_samples, W], BF16, kind="ExternalInput")
    wts = nc.dram_tensor("wts", [128, 3 * n_samples, SH], BF16,
                         kind="ExternalInput")
    out = nc.dram_tensor("out", [H, n_samples, W], BF16,
                         kind="ExternalOutput")

    BW = n_samples * W

    with TileContext(nc) as tc:
        with tc.tile_pool(name="const", bufs=1) as cpool, \
             tc.tile_pool(name="data", bufs=3) as dpool, \
             tc.tile_pool(name="psum", bufs=4, space="PSUM") as ppool:

            # wts rides first on the Scalar HWDGE queue: it must not delay
            # the first x stripe (Sync queue), and SWDGE (GpSimd) emits
            # descriptors ~8x slower. g only feeds blends, which trail the
            # first matmuls anyway.
            scr = cpool.tile([128, W], BF16)
            nc.gpsimd.memset(scr[:], 0.0)
            wt = cpool.tile([128, 3 * n_samples, SH], BF16)
            half_slots = 3 * (n_samples // 2)
            nc.scalar.dma_start(out=wt[:, 0:half_slots, :],
                                in_=wts[:, 0:half_slots, :])
            nc.scalar.dma_start(out=wt[:, half_slots:, :],
                                in_=wts[:, half_slots:, :])

            # PE HAM warm-up: the clock gate starts at ~half rate and needs
            # ~4us of sustained activity to open. Burn zero matmuls on a
            # scratch tile while the first loads are in flight.
            wps = ppool.tile([128, 2 * W], F32, tag="ps", name="warm")
            NWARM = 16
            for i in range(NWARM):
                nc.tensor.matmul(wps[:, 0:W], scr[:, 0:128], scr[:],
                                 start=(i == 0), stop=(i == NWARM - 1))

            for s in (0, 1, 2, 3, 4):
                kdim = TAIL + 2 if s == 4 else 128
                cdim = TAIL if s == 4 else SH

                xs = dpool.tile([128, BW], BF16, tag="xs")
                gs = dpool.tile([128, BW], BF16, tag="gs")
                ol = dpool.tile([128, BW], BF16, tag="ol")

                # stripe tile partition p holds image row SH*s + p - 1
                # (= padded-x row SH*s + p; rows -1 and H are host zeros).
                nc.sync.dma_start(
                    out=xs[0:kdim, :],
                    in_=x[SH * s:SH * s + kdim].rearrange(
                        "p b c -> p (b c)"))
                nc.scalar.dma_start(
                    out=gs[0:cdim, :],
                    in_=g[SH * s:SH * s + cdim].rearrange("p b c -> p (b c)"))

                # dj-major across samples: consecutive matmuls hit different
                # PSUM banks (all 8 in flight), so the engine's reorder
                # window can prefetch LDWEIGHTS and back-to-back matmuls
                # keep the PE array duty high enough that the HAM clock
                # gate stays open.
                pss = [ppool.tile([128, 2 * W], F32, tag="ps",
                                  name=f"ps{s}_{bp}")
                       for bp in range(n_samples // 2)]
                for dji, dj in enumerate((1, 0, 2)):
                    for b in range(n_samples):
                        o = b * W
                        po = (b % 2) * W
                        ps = pss[b // 2][:, po:po + W]
                        if dj == 1:   # center tap: full width, clears
                            nc.tensor.matmul(
                                ps[0:cdim, :], wt[0:kdim, 3 * b + 1, 0:cdim],
                                xs[0:kdim, o:o + W], start=True, stop=False)
                        elif dj == 0:  # out col j taps x col j-1
                            nc.tensor.matmul(
                                ps[0:cdim, 1:W], wt[0:kdim, 3 * b, 0:cdim],
                                xs[0:kdim, o:o + W - 1],
                                start=False, stop=False)
                        else:          # dj=2: out col j taps x col j+1
                            nc.tensor.matmul(
                                ps[0:cdim, 0:W - 1],
                                wt[0:kdim, 3 * b + 2, 0:cdim],
                                xs[0:kdim, o + 1:o + W],
                                start=False, stop=True)

                for bp in range(n_samples // 2):
                    o = 2 * bp * W
                    # blend: out = (x + f/6) + psum, two samples per DVE
                    # op (the pair PSUM tile spans 2 banks) to halve the
                    # per-op fixed overhead; x folded into g host-side.
                    nc.vector.scalar_tensor_tensor(
                        out=ol[0:cdim, o:o + 2 * W],
                        in0=gs[0:cdim, o:o + 2 * W],
                        scalar=1.0, in1=pss[bp][0:cdim, :],
                        op0=ALU.mult, op1=ALU.add)

                nc.gpsimd.dma_start(
                    out=out[SH * s:SH * s + cdim].rearrange(
                        "p b c -> p (b c)"),
                    in_=ol[0:cdim, :])
    return nc


def _make_wts(kA):
    """[128, 24, 126] bf16: slot 3b+dj holds the banded conv weight
    W[p, c] = -kA[b, 0, p-c, dj]/6 (p-c in 0..2)."""
    w = np.zeros((128, 3 * BPC, SH), np.float32)
    c = np.arange(SH)
    for b in range(BPC):
        for dj in range(3):
            for di in range(3):
                w[c + di, 3 * b + dj, c] = -kA[b, 0, di, dj] / 6.0
    return w.astype(NPBF16)


def _make_in_maps(x, f, kernelA):
    in_maps = []
    for cid in range(N_CORES):
        s = slice(cid * BPC, (cid + 1) * BPC)
        # [B, 1, H, W] -> [H+2, B, W]: one row across all samples is 8KB;
        # zero halo rows at top/bottom keep all loads partition-0-based.
        xt = np.zeros((H + 2, BPC, W), dtype=NPBF16)
        xt[1:H + 1] = x[s, 0].transpose(1, 0, 2).astype(NPBF16)
        # g = x + f/6 folded host-side: the blend adds it in one DVE op,
        # so no identity matmul is needed to bring x into PSUM.
        gt = np.ascontiguousarray(
            (x[s, 0] + f[s, 0] * (1.0 / 6.0)).transpose(1, 0, 2)
        ).astype(NPBF16)
        in_maps.append({"x": xt, "g": gt, "wts": _make_wts(kernelA[s])})
    return in_maps


def run_sharded(x, f, kernelA, trace=False):
    """Compile+run on 8 cores; returns (full output, BassKernelResults)."""
    x = np.asarray(x, dtype=np.float32)
    f = np.asarray(f, dtype=np.float32)
    kernelA = np.asarray(kernelA, dtype=np.float32)
    nc = gen_kernel()
    _fixup_sync_waits(nc)
    res = run_bass_kernel_spmd(nc, _make_in_maps(x, f, kernelA),
                               core_ids=list(range(N_CORES)), trace=trace)
    out = np.concatenate(
        [res.results[c]["out"].astype(np.float32)
         .transpose(1, 0, 2).reshape(BPC, 1, H, W)
         for c in range(N_CORES)], axis=0)
    return out, res


def kernel(x, f, kernelA):
    out, _ = run_sharded(x, f, kernelA, trace=False)
    return out


# revision 45
# speedup vs baseline: 1.0097x; 1.0097x over previous
"""Trainium2 Bass kernel for nn_ChebySemi_70222715289681.

out = x + (f - conv3x3(x, kernelA)) / 6   (per-sample 3x3 kernels,
B=64 images of 512x512, fp32). Pure data parallel: batch sharded 8
samples per core across 8 NeuronCores, zero communication.

Per-core kernel (batch-transposed striped layout, bf16 wire format,
106.7us baseline -> ~56us):
  Host ships x TRANSPOSED+row-padded to [H+2, B, W] bf16 and
  g = x + f/6 as [H, B, W] bf16, so one image row across all 8 samples
  is 8KB contiguous in HBM: every stripe DMA moves ~1MB in 8KB
  per-partition descriptors (row-per-partition layouts with 1-2KB
  descriptors measured only ~77-147 GB/s/queue vs ~190-220 here, and
  any partition range not starting at 0 degenerates to ONE SDMA
  engine at ~27 GB/s - hence the host-side zero halo rows).
  The image is processed in 5 row-stripes (4 x 126 output rows + an
  8-row tail); a stripe tile [128, 8*512] holds rows 126s-1..126s+126
  one-row-per-partition. With adjacent rows on adjacent partitions the
  conv's three ROW taps collapse into one banded stationary matrix
  W_dj[p,c] = -k[p-c,dj]/6, so each sample needs only 3 matmuls per
  stripe - the column taps dj ride on shifted PSUM output windows
  (dj=1 full width with start=True, dj=0 into cols 1.., dj=2 into
  cols ..511), which also kills all column padding. x itself never
  passes through the PE: the host fold g = x + f/6 makes the single
  fused DVE blend out = g + psum complete the update. Matmuls issue
  dj-major across samples into 4 two-bank PSUM pair tiles (ILP across
  banks; one wide [126, 1024] blend per pair halves DVE overhead),
  and 16 zero matmuls warm the PE HAM clock gate (starts at ~half
  clock; K=128 activity opens it) while the first loads fly.
  The 24 banded weights are built host-side from kernelA and shipped
  as one [128, 24, 126] bf16 tensor, loaded first on the Scalar ring.
  Loads issue on Sync (x) / Scalar (wts, g) HWDGE rings, stores on
  the GpSimd SWDGE ring so a store waiting on compute never
  head-of-line-blocks a load. Output is stored bf16 [H, B, W]; host
  casts/untransposes to f32 [B,1,H,W]. All wire tensors are bf16
  (13.5MB/core total vs 25.2 in f32; rel err ~2.5e-3, gate 2e-2).
"""
import numpy as np
import concourse.bass as bass
import concourse.mybir as mybir
from concourse.tile import TileContext
from concourse.bass_utils import run_bass_kernel_spmd

F32 = mybir.dt.float32
BF16 = mybir.dt.bfloat16
NPBF16 = mybir.dt.np(BF16)
ALU = mybir.AluOpType

N_CORES = 8
BPC = 8          # samples per core
H = W = 512
SH = 126         # output rows per full stripe
NS = 5           # stripes (4 full + tail)
TAIL = H - 4 * SH  # 8

_MAX_WAITS = 1


def _fixup_sync_waits(nc):
    """This walrus build rejects >1-2 sem-waits per instruction; move the
    excess onto NOPs inserted just before, on the same engine (same program
    order, so semantics are unchanged)."""
    n_fix = 0
    for fn in nc.m.functions:
        for blk in fn.blocks:
            out, changed = [], False
            for inst in blk.instructions:
                si = inst.sync_info
                waits = list(si.on_wait or []) if si is not None else []
                if len(waits) > _MAX_WAITS:
                    changed = True
                    n_fix += 1
                    for i in range(0, len(waits) - _MAX_WAITS, _MAX_WAITS):
                        nop = mybir.InstNoOp(
                            name=f"I-waitfix-{nc.next_id()}", ins=[], outs=[])
                        nop.engine = inst.engine
                        nop.sync_info = mybir.SyncInfo(
                            on_wait=waits[i:i + _MAX_WAITS], on_update=[])
                        out.append(nop)
                    inst.sync_info = mybir.SyncInfo(
                        on_wait=waits[len(waits) - _MAX_WAITS:],
                        on_update=list(si.on_update or []))
                out.append(inst)
            if changed:
                blk.instructions = out
    return n_fix


def gen_kernel(n_samples=BPC):
    nc = bass.Bass(target_bir_lowering=False)
    # x is host-padded with a zero row on top and bottom ([H+2, B, W]) so
    # every stripe load covers a partition range starting at 0: a dst
    # partition range starting elsewhere (e.g. [1:128]) defeats the
    # DGE's per-engine descriptor split - all descriptors land on ONE
    # SDMA engine and the transfer serializes at ~27 GB/s.
    x = nc.dram_tensor("x", [H + 2, n_samples, W], BF16,
                       kind="ExternalInput")
    g = nc.dram_tensor("g", [H, n_samples, W], BF16, kind="ExternalInput")
    wts = nc.dram_tensor("wts", [128, 3 * n_samples + 3, SH], BF16,
                         kind="ExternalInput")
    out = nc.dram_tensor("out", [H, n_samples, W], BF16,
                         kind="ExternalOutput")

    BW = n_samples * W

    with TileContext(nc) as tc:
        with tc.tile_pool(name="const", bufs=1) as cpool, \
             tc.tile_pool(name="data", bufs=3) as dpool, \
             tc.tile_pool(name="psum", bufs=4, space="PSUM") as ppool:

            # wts rides first on the Scalar HWDGE queue: it must not delay
            # the first x stripe (Sync queue), and SWDGE (GpSimd) emits
            # descriptors ~8x slower. g only feeds blends, which trail the
            # first matmuls anyway.
            scr = cpool.tile([128, W], BF16)
            nc.gpsimd.memset(scr[:], 0.0)
            wt = cpool.tile([128, 3 * n_samples + 3, SH], BF16)
            half_slots = 3 * (n_samples // 2)
            nc.scalar.dma_start(out=wt[:, 0:half_slots, :],
                                in_=wts[:, 0:half_slots, :])
            nc.scalar.dma_start(out=wt[:, half_slots:, :],
                                in_=wts[:, half_slots:, :])

            # PE HAM warm-up: the clock gate starts at ~half rate and needs
            # ~4us of sustained activity to open. Burn zero matmuls on a
            # scratch tile while the first loads are in flight.
            wps = ppool.tile([128, 2 * W], F32, tag="ps", name="warm")
            NWARM = 16
            for i in range(NWARM):
                nc.tensor.matmul(wps[:, 0:W], scr[:, 0:128], scr[:],
                                 start=(i == 0), stop=(i == NWARM - 1))

            for s in (0, 1, 2, 3):
                kdim = 128
                cdim = SH

                xs = dpool.tile([128, BW], BF16, tag="xs")
                gs = dpool.tile([128, BW], BF16, tag="gs")
                ol = dpool.tile([128, BW], BF16, tag="ol")

                # stripe tile partition p holds image row SH*s + p - 1
                # (= padded-x row SH*s + p; rows -1 and H are host zeros).
                nc.sync.dma_start(
                    out=xs[0:kdim, :],
                    in_=x[SH * s:SH * s + kdim].rearrange(
                        "p b c -> p (b c)"))
                nc.scalar.dma_start(
                    out=gs[0:cdim, :],
                    in_=g[SH * s:SH * s + cdim].rearrange("p b c -> p (b c)"))

                # dj-major across samples: consecutive matmuls hit different
                # PSUM banks (all 8 in flight), so the engine's reorder
                # window can prefetch LDWEIGHTS and back-to-back matmuls
                # keep the PE array duty high enough that the HAM clock
                # gate stays open.
                pss = [ppool.tile([128, 2 * W], F32, tag="ps",
                                  name=f"ps{s}_{bp}")
                       for bp in range(n_samples // 2)]
                for dji, dj in enumerate((1, 0, 2)):
                    for b in range(n_samples):
                        o = b * W
                        po = (b % 2) * W
                        ps = pss[b // 2][:, po:po + W]
                        if dj == 1:   # center tap: full width, clears
                            nc.tensor.matmul(
                                ps[0:cdim, :], wt[0:kdim, 3 * b + 1, 0:cdim],
                                xs[0:kdim, o:o + W], start=True, stop=False)
                        elif dj == 0:  # out col j taps x col j-1
                            nc.tensor.matmul(
                                ps[0:cdim, 1:W], wt[0:kdim, 3 * b, 0:cdim],
                                xs[0:kdim, o:o + W - 1],
                                start=False, stop=False)
                        else:          # dj=2: out col j taps x col j+1
                            nc.tensor.matmul(
                                ps[0:cdim, 0:W - 1],
                                wt[0:kdim, 3 * b + 2, 0:cdim],
                                xs[0:kdim, o + 1:o + W],
                                start=False, stop=True)

                for bp in range(n_samples // 2):
                    o = 2 * bp * W
                    # blend: out = (x + f/6) + psum, two samples per DVE
                    # op (the pair PSUM tile spans 2 banks) to halve the
                    # per-op fixed overhead; x folded into g host-side.
                    nc.vector.scalar_tensor_tensor(
                        out=ol[0:cdim, o:o + 2 * W],
                        in0=gs[0:cdim, o:o + 2 * W],
                        scalar=1.0, in1=pss[bp][0:cdim, :],
                        op0=ALU.mult, op1=ALU.add)

                nc.gpsimd.dma_start(
                    out=out[SH * s:SH * s + cdim].rearrange(
                        "p b c -> p (b c)"),
                    in_=ol[0:cdim, :])

            # Tail (last 8 rows of every sample) as ONE block-diagonal
            # stripe: partition 10b+r holds padded row 504+r of sample b,
            # so W_tail[80, 64] (8 diagonal [10, 8] bands, one per sample)
            # computes all samples' tail rows in 3 matmuls + 1 blend
            # instead of 24 matmuls + 4 blends.
            KT = 10 * n_samples   # 80 input partitions
            CT = TAIL * n_samples  # 64 output partitions
            xs = dpool.tile([128, BW], BF16, tag="xs")
            gs = dpool.tile([128, BW], BF16, tag="gs")
            ol = dpool.tile([128, BW], BF16, tag="ol")
            xap = x[4 * SH:4 * SH + 10]
            nc.sync.dma_start(
                out=xs[0:KT, 0:W],
                in_=bass.AP(xap.tensor, xap.offset,
                            [[W, n_samples], [BW, 10], [1, W]]))
            gap = g[4 * SH:H]
            nc.scalar.dma_start(
                out=gs[0:CT, 0:W],
                in_=bass.AP(gap.tensor, gap.offset,
                            [[W, n_samples], [BW, TAIL], [1, W]]))
            ps = ppool.tile([128, 2 * W], F32, tag="ps", name="ps_tail")
            TS = 3 * n_samples  # first tail weight slot
            nc.tensor.matmul(ps[0:CT, 0:W], wt[0:KT, TS + 1, 0:CT],
                             xs[0:KT, 0:W], start=True, stop=False)
            nc.tensor.matmul(ps[0:CT, 1:W], wt[0:KT, TS, 0:CT],
                             xs[0:KT, 0:W - 1], start=False, stop=False)
            nc.tensor.matmul(ps[0:CT, 0:W - 1], wt[0:KT, TS + 2, 0:CT],
                             xs[0:KT, 1:W], start=False, stop=True)
            nc.vector.scalar_tensor_tensor(
                out=ol[0:CT, 0:W], in0=gs[0:CT, 0:W], scalar=1.0,
                in1=ps[0:CT, 0:W], op0=ALU.mult, op1=ALU.add)
            oap = out[4 * SH:H]
            nc.gpsimd.dma_start(
                out=bass.AP(oap.tensor, oap.offset,
                            [[W, n_samples], [BW, TAIL], [1, W]]),
                in_=ol[0:CT, 0:W])
    return nc


def _make_wts(kA):
    """[128, 27, 126] bf16: slot 3b+dj holds the banded conv weight
    W[p, c] = -kA[b, 0, p-c, dj]/6 (p-c in 0..2); slots 24..26 hold the
    block-diagonal tail weights W[10b+r+di, 8b+r] = -kA[b, 0, di, dj]/6
    that compute every sample's last 8 rows in one matmul per dj."""
    w = np.zeros((128, 3 * BPC + 3, SH), np.float32)
    c = np.arange(SH)
    r = np.arange(TAIL)
    for b in range(BPC):
        for dj in range(3):
            for di in range(3):
                w[c + di, 3 * b + dj, c] = -kA[b, 0, di, dj] / 6.0
                w[10 * b + r + di, 3 * BPC + dj, TAIL * b + r] = \
                    -kA[b, 0, di, dj] / 6.0
    return w.astype(NPBF16)


def _make_in_maps(x, f, kernelA):
    in_maps = []
    for cid in range(N_CORES):
        s = slice(cid * BPC, (cid + 1) * BPC)
        # [B, 1, H, W] -> [H+2, B, W]: one row across all samples is 8KB;
        # zero halo rows at top/bottom keep all loads partition-0-based.
        xt = np.zeros((H + 2, BPC, W), dtype=NPBF16)
        xt[1:H + 1] = x[s, 0].transpose(1, 0, 2).astype(NPBF16)
        # g = x + f/6 folded host-side: the blend adds it in one DVE op,
        # so no identity matmul is needed to bring x into PSUM.
        gt = np.ascontiguousarray(
            (x[s, 0] + f[s, 0] * (1.0 / 6.0)).transpose(1, 0, 2)
        ).astype(NPBF16)
        in_maps.append({"x": xt, "g": gt, "wts": _make_wts(kernelA[s])})
    return in_maps


def run_sharded(x, f, kernelA, trace=False):
    """Compile+run on 8 cores; returns (full output, BassKernelResults)."""
    x = np.asarray(x, dtype=np.float32)
    f = np.asarray(f, dtype=np.float32)
    kernelA = np.asarray(kernelA, dtype=np.float32)
    nc = gen_kernel()
    _fixup_sync_waits(nc)
    res = run_bass_kernel_spmd(nc, _make_in_maps(x, f, kernelA),
                               core_ids=list(range(N_CORES)), trace=trace)
    out = np.concatenate(
        [res.results[c]["out"].astype(np.float32)
         .transpose(1, 0, 2).reshape(BPC, 1, H, W)
         for c in range(N_CORES)], axis=0)
    return out, res


def kernel(x, f, kernelA):
    out, _ = run_sharded(x, f, kernelA, trace=False)
    return out


# revision 46
# speedup vs baseline: 1.0751x; 1.0648x over previous
"""Trainium2 Bass kernel for nn_ChebySemi_70222715289681.

out = x + (f - conv3x3(x, kernelA)) / 6   (per-sample 3x3 kernels,
B=64 images of 512x512, fp32). Pure data parallel: batch sharded 8
samples per core across 8 NeuronCores, zero communication.

Per-core kernel (batch-transposed striped layout, bf16 wire format,
106.7us baseline -> ~56us):
  Host ships x TRANSPOSED+row-padded to [H+2, B, W] bf16 and
  g = x + f/6 as [H, B, W] bf16, so one image row across all 8 samples
  is 8KB contiguous in HBM: every stripe DMA moves ~1MB in 8KB
  per-partition descriptors (row-per-partition layouts with 1-2KB
  descriptors measured only ~77-147 GB/s/queue vs ~190-220 here, and
  any partition range not starting at 0 degenerates to ONE SDMA
  engine at ~27 GB/s - hence the host-side zero halo rows).
  The image is processed in 5 row-stripes (4 x 126 output rows + an
  8-row tail); a stripe tile [128, 8*512] holds rows 126s-1..126s+126
  one-row-per-partition. With adjacent rows on adjacent partitions the
  conv's three ROW taps collapse into one banded stationary matrix
  W_dj[p,c] = -k[p-c,dj]/6, so each sample needs only 3 matmuls per
  stripe - the column taps dj ride on shifted PSUM output windows
  (dj=1 full width with start=True, dj=0 into cols 1.., dj=2 into
  cols ..511), which also kills all column padding. x itself never
  passes through the PE: the host fold g = x + f/6 makes the single
  fused DVE blend out = g + psum complete the update. Matmuls issue
  dj-major across samples into 4 two-bank PSUM pair tiles (ILP across
  banks; one wide [126, 1024] blend per pair halves DVE overhead),
  and 16 zero matmuls warm the PE HAM clock gate (starts at ~half
  clock; K=128 activity opens it) while the first loads fly.
  The 24 banded weights are built host-side from kernelA and shipped
  as one [128, 24, 126] bf16 tensor, loaded first on the Scalar ring.
  Loads issue on Sync (x) / Scalar (wts, g) HWDGE rings, stores on
  the GpSimd SWDGE ring so a store waiting on compute never
  head-of-line-blocks a load. Output is stored bf16 [H, B, W]; host
  casts/untransposes to f32 [B,1,H,W]. All wire tensors are bf16
  (13.5MB/core total vs 25.2 in f32; rel err ~2.5e-3, gate 2e-2).
"""
import numpy as np
import concourse.bass as bass
import concourse.mybir as mybir
from concourse.tile import TileContext
from concourse.bass_utils import run_bass_kernel_spmd

F32 = mybir.dt.float32
BF16 = mybir.dt.bfloat16
NPBF16 = mybir.dt.np(BF16)
ALU = mybir.AluOpType

N_CORES = 8
BPC = 8          # samples per core
H = W = 512
SH = 126         # output rows per full stripe
NS = 5           # stripes (4 full + tail)
TAIL = H - 4 * SH  # 8

_MAX_WAITS = 1


def _fixup_sync_waits(nc):
    """This walrus build rejects >1-2 sem-waits per instruction; move the
    excess onto NOPs inserted just before, on the same engine (same program
    order, so semantics are unchanged)."""
    n_fix = 0
    for fn in nc.m.functions:
        for blk in fn.blocks:
            out, changed = [], False
            for inst in blk.instructions:
                si = inst.sync_info
                waits = list(si.on_wait or []) if si is not None else []
                if len(waits) > _MAX_WAITS:
                    changed = True
                    n_fix += 1
                    for i in range(0, len(waits) - _MAX_WAITS, _MAX_WAITS):
                        nop = mybir.InstNoOp(
                            name=f"I-waitfix-{nc.next_id()}", ins=[], outs=[])
                        nop.engine = inst.engine
                        nop.sync_info = mybir.SyncInfo(
                            on_wait=waits[i:i + _MAX_WAITS], on_update=[])
                        out.append(nop)
                    inst.sync_info = mybir.SyncInfo(
                        on_wait=waits[len(waits) - _MAX_WAITS:],
                        on_update=list(si.on_update or []))
                out.append(inst)
            if changed:
                blk.instructions = out
    return n_fix


def gen_kernel(n_samples=BPC):
    nc = bass.Bass(target_bir_lowering=False)
    # x is host-padded with a zero row on top and bottom ([H+2, B, W]) so
    # every stripe load covers a partition range starting at 0: a dst
    # partition range starting elsewhere (e.g. [1:128]) defeats the
    # DGE's per-engine descriptor split - all descriptors land on ONE
    # SDMA engine and the transfer serializes at ~27 GB/s.
    x = nc.dram_tensor("x", [H + 2, n_samples, W], BF16,
                       kind="ExternalInput")
    g = nc.dram_tensor("g", [H, n_samples, W], BF16, kind="ExternalInput")
    wts = nc.dram_tensor("wts", [128, 3 * n_samples + 3, SH], BF16,
                         kind="ExternalInput")
    out = nc.dram_tensor("out", [H, n_samples, W], BF16,
                         kind="ExternalOutput")

    BW = n_samples * W

    with TileContext(nc) as tc:
        with tc.tile_pool(name="const", bufs=1) as cpool, \
             tc.tile_pool(name="data", bufs=3) as dpool, \
             tc.tile_pool(name="psum", bufs=4, space="PSUM") as ppool:

            # wts rides first on the Scalar HWDGE queue: it must not delay
            # the first x stripe (Sync queue), and SWDGE (GpSimd) emits
            # descriptors ~8x slower. g only feeds blends, which trail the
            # first matmuls anyway.
            scr = cpool.tile([128, W], BF16)
            nc.gpsimd.memset(scr[:], 0.0)
            wt = cpool.tile([128, 3 * n_samples + 3, SH], BF16)
            half_slots = 3 * (n_samples // 2)
            nc.scalar.dma_start(out=wt[:, 0:half_slots, :],
                                in_=wts[:, 0:half_slots, :])
            nc.scalar.dma_start(out=wt[:, half_slots:, :],
                                in_=wts[:, half_slots:, :])

            # PE HAM warm-up: the clock gate starts at ~half rate and needs
            # ~4us of sustained activity to open. Burn zero matmuls on a
            # scratch tile while the first loads are in flight.
            wps = ppool.tile([128, 2 * W], F32, tag="ps", name="warm")
            NWARM = 24
            for i in range(NWARM):
                nc.tensor.matmul(wps[:, 0:W], scr[:, 0:128], scr[:],
                                 start=(i == 0), stop=(i == NWARM - 1))

            for s in (0, 1, 2, 3):
                kdim = 128
                cdim = SH

                xs = dpool.tile([128, BW], BF16, tag="xs")
                gs = dpool.tile([128, BW], BF16, tag="gs")
                ol = dpool.tile([128, BW], BF16, tag="ol")

                # stripe tile partition p holds image row SH*s + p - 1
                # (= padded-x row SH*s + p; rows -1 and H are host zeros).
                nc.sync.dma_start(
                    out=xs[0:kdim, :],
                    in_=x[SH * s:SH * s + kdim].rearrange(
                        "p b c -> p (b c)"))
                nc.scalar.dma_start(
                    out=gs[0:cdim, :],
                    in_=g[SH * s:SH * s + cdim].rearrange("p b c -> p (b c)"))

                # dj-major across samples: consecutive matmuls hit different
                # PSUM banks (all 8 in flight), so the engine's reorder
                # window can prefetch LDWEIGHTS and back-to-back matmuls
                # keep the PE array duty high enough that the HAM clock
                # gate stays open.
                pss = [ppool.tile([128, 2 * W], F32, tag="ps",
                                  name=f"ps{s}_{bp}")
                       for bp in range(n_samples // 2)]
                for dji, dj in enumerate((1, 0, 2)):
                    for b in range(n_samples):
                        o = b * W
                        po = (b % 2) * W
                        ps = pss[b // 2][:, po:po + W]
                        if dj == 1:   # center tap: full width, clears
                            nc.tensor.matmul(
                                ps[0:cdim, :], wt[0:kdim, 3 * b + 1, 0:cdim],
                                xs[0:kdim, o:o + W], start=True, stop=False)
                        elif dj == 0:  # out col j taps x col j-1
                            nc.tensor.matmul(
                                ps[0:cdim, 1:W], wt[0:kdim, 3 * b, 0:cdim],
                                xs[0:kdim, o:o + W - 1],
                                start=False, stop=False)
                        else:          # dj=2: out col j taps x col j+1
                            nc.tensor.matmul(
                                ps[0:cdim, 0:W - 1],
                                wt[0:kdim, 3 * b + 2, 0:cdim],
                                xs[0:kdim, o + 1:o + W],
                                start=False, stop=True)

                for bp in range(n_samples // 2):
                    o = 2 * bp * W
                    # blend: out = (x + f/6) + psum, two samples per DVE
                    # op (the pair PSUM tile spans 2 banks) to halve the
                    # per-op fixed overhead; x folded into g host-side.
                    nc.vector.scalar_tensor_tensor(
                        out=ol[0:cdim, o:o + 2 * W],
                        in0=gs[0:cdim, o:o + 2 * W],
                        scalar=1.0, in1=pss[bp][0:cdim, :],
                        op0=ALU.mult, op1=ALU.add)

                if s == 3:
                    # split the final full-stripe store so its 1MB drain
                    # starts as soon as the first two pairs are blended
                    for hh in range(2):
                        nc.gpsimd.dma_start(
                            out=out[SH * s:SH * s + cdim,
                                    4 * hh:4 * hh + 4, :].rearrange(
                                "p b c -> p (b c)"),
                            in_=ol[0:cdim, 4 * hh * W:(4 * hh + 4) * W])
                else:
                    nc.gpsimd.dma_start(
                        out=out[SH * s:SH * s + cdim].rearrange(
                            "p b c -> p (b c)"),
                        in_=ol[0:cdim, :])

            # Tail (last 8 rows of every sample) as ONE block-diagonal
            # stripe: partition 10b+r holds padded row 504+r of sample b,
            # so W_tail[80, 64] (8 diagonal [10, 8] bands, one per sample)
            # computes all samples' tail rows in 3 matmuls + 1 blend
            # instead of 24 matmuls + 4 blends.
            KT = 10 * n_samples   # 80 input partitions
            CT = TAIL * n_samples  # 64 output partitions
            xs = dpool.tile([128, BW], BF16, tag="xs")
            gs = dpool.tile([128, BW], BF16, tag="gs")
            ol = dpool.tile([128, BW], BF16, tag="ol")
            xap = x[4 * SH:4 * SH + 10]
            nc.sync.dma_start(
                out=xs[0:KT, 0:W],
                in_=bass.AP(xap.tensor, xap.offset,
                            [[W, n_samples], [BW, 10], [1, W]]))
            gap = g[4 * SH:H]
            nc.scalar.dma_start(
                out=gs[0:CT, 0:W],
                in_=bass.AP(gap.tensor, gap.offset,
                            [[W, n_samples], [BW, TAIL], [1, W]]))
            ps = ppool.tile([128, 2 * W], F32, tag="ps", name="ps_tail")
            TS = 3 * n_samples  # first tail weight slot
            nc.tensor.matmul(ps[0:CT, 0:W], wt[0:KT, TS + 1, 0:CT],
                             xs[0:KT, 0:W], start=True, stop=False)
            nc.tensor.matmul(ps[0:CT, 1:W], wt[0:KT, TS, 0:CT],
                             xs[0:KT, 0:W - 1], start=False, stop=False)
            nc.tensor.matmul(ps[0:CT, 0:W - 1], wt[0:KT, TS + 2, 0:CT],
                             xs[0:KT, 1:W], start=False, stop=True)
            nc.vector.scalar_tensor_tensor(
                out=ol[0:CT, 0:W], in0=gs[0:CT, 0:W], scalar=1.0,
                in1=ps[0:CT, 0:W], op0=ALU.mult, op1=ALU.add)
            oap = out[4 * SH:H]
            nc.gpsimd.dma_start(
                out=bass.AP(oap.tensor, oap.offset,
                            [[W, n_samples], [BW, TAIL], [1, W]]),
                in_=ol[0:CT, 0:W])
    return nc


def _make_wts(kA):
    """[128, 27, 126] bf16: slot 3b+dj holds the banded conv weight
    W[p, c] = -kA[b, 0, p-c, dj]/6 (p-c in 0..2); slots 24..26 hold the
    block-diagonal tail weights W[10b+r+di, 8b+r] = -kA[b, 0, di, dj]/6
    that compute every sample's last 8 rows in one matmul per dj."""
    w = np.zeros((128, 3 * BPC + 3, SH), np.float32)
    c = np.arange(SH)
    r = np.arange(TAIL)
    for b in range(BPC):
        for dj in range(3):
            for di in range(3):
                w[c + di, 3 * b + dj, c] = -kA[b, 0, di, dj] / 6.0
                w[10 * b + r + di, 3 * BPC + dj, TAIL * b + r] = \
                    -kA[b, 0, di, dj] / 6.0
    return w.astype(NPBF16)


def _make_in_maps(x, f, kernelA):
    in_maps = []
    for cid in range(N_CORES):
        s = slice(cid * BPC, (cid + 1) * BPC)
        # [B, 1, H, W] -> [H+2, B, W]: one row across all samples is 8KB;
        # zero halo rows at top/bottom keep all loads partition-0-based.
        xt = np.zeros((H + 2, BPC, W), dtype=NPBF16)
        xt[1:H + 1] = x[s, 0].transpose(1, 0, 2).astype(NPBF16)
        # g = x + f/6 folded host-side: the blend adds it in one DVE op,
        # so no identity matmul is needed to bring x into PSUM.
        gt = np.ascontiguousarray(
            (x[s, 0] + f[s, 0] * (1.0 / 6.0)).transpose(1, 0, 2)
        ).astype(NPBF16)
        in_maps.append({"x": xt, "g": gt, "wts": _make_wts(kernelA[s])})
    return in_maps


def run_sharded(x, f, kernelA, trace=False):
    """Compile+run on 8 cores; returns (full output, BassKernelResults)."""
    x = np.asarray(x, dtype=np.float32)
    f = np.asarray(f, dtype=np.float32)
    kernelA = np.asarray(kernelA, dtype=np.float32)
    nc = gen_kernel()
    _fixup_sync_waits(nc)
    res = run_bass_kernel_spmd(nc, _make_in_maps(x, f, kernelA),
                               core_ids=list(range(N_CORES)), trace=trace)
    out = np.concatenate(
        [res.results[c]["out"].astype(np.float32)
         .transpose(1, 0, 2).reshape(BPC, 1, H, W)
         for c in range(N_CORES)], axis=0)
    return out, res


def kernel(x, f, kernelA):
    out, _ = run_sharded(x, f, kernelA, trace=False)
    return out


# revision 47
# speedup vs baseline: 1.1172x; 1.0392x over previous
"""Trainium2 Bass kernel for nn_ChebySemi_70222715289681.

out = x + (f - conv3x3(x, kernelA)) / 6   (per-sample 3x3 kernels,
B=64 images of 512x512, fp32). Pure data parallel: batch sharded 8
samples per core across 8 NeuronCores, zero communication.

Per-core kernel (batch-transposed striped layout, bf16 wire format,
106.7us baseline -> ~56us):
  Host ships x TRANSPOSED+row-padded to [H+2, B, W] bf16 and
  g = x + f/6 as [H, B, W] bf16, so one image row across all 8 samples
  is 8KB contiguous in HBM: every stripe DMA moves ~1MB in 8KB
  per-partition descriptors (row-per-partition layouts with 1-2KB
  descriptors measured only ~77-147 GB/s/queue vs ~190-220 here, and
  any partition range not starting at 0 degenerates to ONE SDMA
  engine at ~27 GB/s - hence the host-side zero halo rows).
  The image is processed in 5 row-stripes (4 x 126 output rows + an
  8-row tail); a stripe tile [128, 8*512] holds rows 126s-1..126s+126
  one-row-per-partition. With adjacent rows on adjacent partitions the
  conv's three ROW taps collapse into one banded stationary matrix
  W_dj[p,c] = -k[p-c,dj]/6, so each sample needs only 3 matmuls per
  stripe - the column taps dj ride on shifted PSUM output windows
  (dj=1 full width with start=True, dj=0 into cols 1.., dj=2 into
  cols ..511), which also kills all column padding. x itself never
  passes through the PE: the host fold g = x + f/6 makes the single
  fused DVE blend out = g + psum complete the update. Matmuls issue
  dj-major across samples into 4 two-bank PSUM pair tiles (ILP across
  banks; one wide [126, 1024] blend per pair halves DVE overhead),
  and 16 zero matmuls warm the PE HAM clock gate (starts at ~half
  clock; K=128 activity opens it) while the first loads fly.
  The 24 banded weights are built host-side from kernelA and shipped
  as one [128, 24, 126] bf16 tensor, loaded first on the Scalar ring.
  Loads issue on Sync (x) / Scalar (wts, g) HWDGE rings, stores on
  the GpSimd SWDGE ring so a store waiting on compute never
  head-of-line-blocks a load. Output is stored bf16 [H, B, W]; host
  casts/untransposes to f32 [B,1,H,W]. All wire tensors are bf16
  (13.5MB/core total vs 25.2 in f32; rel err ~2.5e-3, gate 2e-2).
"""
import numpy as np
import concourse.bass as bass
import concourse.mybir as mybir
from concourse.tile import TileContext
from concourse.bass_utils import run_bass_kernel_spmd

F32 = mybir.dt.float32
BF16 = mybir.dt.bfloat16
NPBF16 = mybir.dt.np(BF16)
ALU = mybir.AluOpType

N_CORES = 8
BPC = 8          # samples per core
H = W = 512
SH = 126         # output rows per full stripe
NS = 5           # stripes (4 full + tail)
TAIL = H - 4 * SH  # 8

_MAX_WAITS = 1


def _fixup_sync_waits(nc):
    """This walrus build rejects >1-2 sem-waits per instruction; move the
    excess onto NOPs inserted just before, on the same engine (same program
    order, so semantics are unchanged)."""
    n_fix = 0
    for fn in nc.m.functions:
        for blk in fn.blocks:
            out, changed = [], False
            for inst in blk.instructions:
                si = inst.sync_info
                waits = list(si.on_wait or []) if si is not None else []
                if len(waits) > _MAX_WAITS:
                    changed = True
                    n_fix += 1
                    for i in range(0, len(waits) - _MAX_WAITS, _MAX_WAITS):
                        nop = mybir.InstNoOp(
                            name=f"I-waitfix-{nc.next_id()}", ins=[], outs=[])
                        nop.engine = inst.engine
                        nop.sync_info = mybir.SyncInfo(
                            on_wait=waits[i:i + _MAX_WAITS], on_update=[])
                        out.append(nop)
                    inst.sync_info = mybir.SyncInfo(
                        on_wait=waits[len(waits) - _MAX_WAITS:],
                        on_update=list(si.on_update or []))
                out.append(inst)
            if changed:
                blk.instructions = out
    return n_fix


def gen_kernel(n_samples=BPC):
    nc = bass.Bass(target_bir_lowering=False)
    # x is host-padded with a zero row on top and bottom ([H+2, B, W]) so
    # every stripe load covers a partition range starting at 0: a dst
    # partition range starting elsewhere (e.g. [1:128]) defeats the
    # DGE's per-engine descriptor split - all descriptors land on ONE
    # SDMA engine and the transfer serializes at ~27 GB/s.
    x = nc.dram_tensor("x", [H + 2, n_samples, W], BF16,
                       kind="ExternalInput")
    g = nc.dram_tensor("g", [H, n_samples, W], BF16, kind="ExternalInput")
    wts = nc.dram_tensor("wts", [128, 3 * n_samples + 3, SH], BF16,
                         kind="ExternalInput")
    out = nc.dram_tensor("out", [H, n_samples, W], BF16,
                         kind="ExternalOutput")

    BW = n_samples * W

    with TileContext(nc) as tc:
        with tc.tile_pool(name="const", bufs=1) as cpool, \
             tc.tile_pool(name="data", bufs=3) as dpool, \
             tc.tile_pool(name="psum", bufs=3, space="PSUM") as ppool, \
             tc.tile_pool(name="warmp", bufs=1, space="PSUM") as wpool:

            # wts rides first on the Scalar HWDGE queue: it must not delay
            # the first x stripe (Sync queue), and SWDGE (GpSimd) emits
            # descriptors ~8x slower. g only feeds blends, which trail the
            # first matmuls anyway.
            scr = cpool.tile([128, W], BF16)
            nc.gpsimd.memset(scr[:], 0.0)
            wt = cpool.tile([128, 3 * n_samples + 3, SH], BF16)
            half_slots = 3 * (n_samples // 2)
            nc.scalar.dma_start(out=wt[:, 0:half_slots, :],
                                in_=wts[:, 0:half_slots, :])
            nc.scalar.dma_start(out=wt[:, half_slots:, :],
                                in_=wts[:, half_slots:, :])

            # PE HAM warm-up: the clock gate starts at ~half rate and needs
            # ~4us of sustained activity to open. Burn zero matmuls on a
            # scratch tile while the first loads are in flight.
            wps = wpool.tile([128, 2 * W], F32, tag="warm", name="warm")
            NWARM = 24
            for i in range(NWARM):
                nc.tensor.matmul(wps[:, 0:W], scr[:, 0:128], scr[:],
                                 start=(i == 0), stop=(i == NWARM - 1))

            for s in (0, 1, 2, 3):
                kdim = 128
                cdim = SH

                xs = dpool.tile([128, BW], BF16, tag="xs")
                gs = dpool.tile([128, BW], BF16, tag="gs")
                ol = dpool.tile([128, BW], BF16, tag="ol")

                # stripe tile partition p holds image row SH*s + p - 1
                # (= padded-x row SH*s + p; rows -1 and H are host zeros).
                nc.sync.dma_start(
                    out=xs[0:kdim, :],
                    in_=x[SH * s:SH * s + kdim].rearrange(
                        "p b c -> p (b c)"))
                nc.scalar.dma_start(
                    out=gs[0:cdim, :],
                    in_=g[SH * s:SH * s + cdim].rearrange("p b c -> p (b c)"))

                # dj-major across samples: consecutive matmuls hit different
                # PSUM banks (all 8 in flight), so the engine's reorder
                # window can prefetch LDWEIGHTS and back-to-back matmuls
                # keep the PE array duty high enough that the HAM clock
                # gate stays open.
                pss = [ppool.tile([128, 2 * W], F32, tag="ps",
                                  name=f"ps{s}_{bp}")
                       for bp in range(n_samples // 2)]
                for dji, dj in enumerate((1, 0, 2)):
                    for b in range(n_samples):
                        o = b * W
                        po = (b % 2) * W
                        ps = pss[b // 2][:, po:po + W]
                        if dj == 1:   # center tap: full width, clears
                            nc.tensor.matmul(
                                ps[0:cdim, :], wt[0:kdim, 3 * b + 1, 0:cdim],
                                xs[0:kdim, o:o + W], start=True, stop=False)
                        elif dj == 0:  # out col j taps x col j-1
                            nc.tensor.matmul(
                                ps[0:cdim, 1:W], wt[0:kdim, 3 * b, 0:cdim],
                                xs[0:kdim, o:o + W - 1],
                                start=False, stop=False)
                        else:          # dj=2: out col j taps x col j+1
                            nc.tensor.matmul(
                                ps[0:cdim, 0:W - 1],
                                wt[0:kdim, 3 * b + 2, 0:cdim],
                                xs[0:kdim, o + 1:o + W],
                                start=False, stop=True)

                for bp in range(n_samples // 2):
                    o = 2 * bp * W
                    # blend: out = (x + f/6) + psum, two samples per DVE
                    # op (the pair PSUM tile spans 2 banks) to halve the
                    # per-op fixed overhead; x folded into g host-side.
                    nc.vector.scalar_tensor_tensor(
                        out=ol[0:cdim, o:o + 2 * W],
                        in0=gs[0:cdim, o:o + 2 * W],
                        scalar=1.0, in1=pss[bp][0:cdim, :],
                        op0=ALU.mult, op1=ALU.add)

                if s == 3:
                    # split the final full-stripe store so its 1MB drain
                    # starts as soon as the first two pairs are blended
                    for hh in range(2):
                        nc.gpsimd.dma_start(
                            out=out[SH * s:SH * s + cdim,
                                    4 * hh:4 * hh + 4, :].rearrange(
                                "p b c -> p (b c)"),
                            in_=ol[0:cdim, 4 * hh * W:(4 * hh + 4) * W])
                else:
                    nc.gpsimd.dma_start(
                        out=out[SH * s:SH * s + cdim].rearrange(
                            "p b c -> p (b c)"),
                        in_=ol[0:cdim, :])

            # Tail (last 8 rows of every sample) as ONE block-diagonal
            # stripe: partition 10b+r holds padded row 504+r of sample b,
            # so W_tail[80, 64] (8 diagonal [10, 8] bands, one per sample)
            # computes all samples' tail rows in 3 matmuls + 1 blend
            # instead of 24 matmuls + 4 blends.
            KT = 10 * n_samples   # 80 input partitions
            CT = TAIL * n_samples  # 64 output partitions
            xs = dpool.tile([128, BW], BF16, tag="xs")
            gs = dpool.tile([128, BW], BF16, tag="gs")
            ol = dpool.tile([128, BW], BF16, tag="ol")
            xap = x[4 * SH:4 * SH + 10]
            nc.sync.dma_start(
                out=xs[0:KT, 0:W],
                in_=bass.AP(xap.tensor, xap.offset,
                            [[W, n_samples], [BW, 10], [1, W]]))
            gap = g[4 * SH:H]
            nc.scalar.dma_start(
                out=gs[0:CT, 0:W],
                in_=bass.AP(gap.tensor, gap.offset,
                            [[W, n_samples], [BW, TAIL], [1, W]]))
            ps = ppool.tile([128, 2 * W], F32, tag="ps", name="ps_tail")
            TS = 3 * n_samples  # first tail weight slot
            nc.tensor.matmul(ps[0:CT, 0:W], wt[0:KT, TS + 1, 0:CT],
                             xs[0:KT, 0:W], start=True, stop=False)
            nc.tensor.matmul(ps[0:CT, 1:W], wt[0:KT, TS, 0:CT],
                             xs[0:KT, 0:W - 1], start=False, stop=False)
            nc.tensor.matmul(ps[0:CT, 0:W - 1], wt[0:KT, TS + 2, 0:CT],
                             xs[0:KT, 1:W], start=False, stop=True)
            nc.vector.scalar_tensor_tensor(
                out=ol[0:CT, 0:W], in0=gs[0:CT, 0:W], scalar=1.0,
                in1=ps[0:CT, 0:W], op0=ALU.mult, op1=ALU.add)
            oap = out[4 * SH:H]
            nc.gpsimd.dma_start(
                out=bass.AP(oap.tensor, oap.offset,
                            [[W, n_samples], [BW, TAIL], [1, W]]),
                in_=ol[0:CT, 0:W])
    return nc


def _make_wts(kA):
    """[128, 27, 126] bf16: slot 3b+dj holds the banded conv weight
    W[p, c] = -kA[b, 0, p-c, dj]/6 (p-c in 0..2); slots 24..26 hold the
    block-diagonal tail weights W[10b+r+di, 8b+r] = -kA[b, 0, di, dj]/6
    that compute every sample's last 8 rows in one matmul per dj."""
    w = np.zeros((128, 3 * BPC + 3, SH), np.float32)
    c = np.arange(SH)
    r = np.arange(TAIL)
    for b in range(BPC):
        for dj in range(3):
            for di in range(3):
                w[c + di, 3 * b + dj, c] = -kA[b, 0, di, dj] / 6.0
                w[10 * b + r + di, 3 * BPC + dj, TAIL * b + r] = \
                    -kA[b, 0, di, dj] / 6.0
    return w.astype(NPBF16)


def _make_in_maps(x, f, kernelA):
    in_maps = []
    for cid in range(N_CORES):
        s = slice(cid * BPC, (cid + 1) * BPC)
        # [B, 1, H, W] -> [H+2, B, W]: one row across all samples is 8KB;
        # zero halo rows at top/bottom keep all loads partition-0-based.
        xt = np.zeros((H + 2, BPC, W), dtype=NPBF16)
        xt[1:H + 1] = x[s, 0].transpose(1, 0, 2).astype(NPBF16)
        # g = x + f/6 folded host-side: the blend adds it in one DVE op,
        # so no identity matmul is needed to bring x into PSUM.
        gt = np.ascontiguousarray(
            (x[s, 0] + f[s, 0] * (1.0 / 6.0)).transpose(1, 0, 2)
        ).astype(NPBF16)
        in_maps.append({"x": xt, "g": gt, "wts": _make_wts(kernelA[s])})
    return in_maps


def run_sharded(x, f, kernelA, trace=False):
    """Compile+run on 8 cores; returns (full output, BassKernelResults)."""
    x = np.asarray(x, dtype=np.float32)
    f = np.asarray(f, dtype=np.float32)
    kernelA = np.asarray(kernelA, dtype=np.float32)
    nc = gen_kernel()
    _fixup_sync_waits(nc)
    res = run_bass_kernel_spmd(nc, _make_in_maps(x, f, kernelA),
                               core_ids=list(range(N_CORES)), trace=trace)
    out = np.concatenate(
        [res.results[c]["out"].astype(np.float32)
         .transpose(1, 0, 2).reshape(BPC, 1, H, W)
         for c in range(N_CORES)], axis=0)
    return out, res


def kernel(x, f, kernelA):
    out, _ = run_sharded(x, f, kernelA, trace=False)
    return out


# revision 48
# speedup vs baseline: 1.1233x; 1.0054x over previous
"""Trainium2 Bass kernel for nn_ChebySemi_70222715289681.

out = x + (f - conv3x3(x, kernelA)) / 6   (per-sample 3x3 kernels,
B=64 images of 512x512, fp32). Pure data parallel: batch sharded 8
samples per core across 8 NeuronCores, zero communication.

Per-core kernel (batch-transposed striped layout, bf16 wire format,
106.7us baseline -> ~56us):
  Host ships x TRANSPOSED+row-padded to [H+2, B, W] bf16 and
  g = x + f/6 as [H, B, W] bf16, so one image row across all 8 samples
  is 8KB contiguous in HBM: every stripe DMA moves ~1MB in 8KB
  per-partition descriptors (row-per-partition layouts with 1-2KB
  descriptors measured only ~77-147 GB/s/queue vs ~190-220 here, and
  any partition range not starting at 0 degenerates to ONE SDMA
  engine at ~27 GB/s - hence the host-side zero halo rows).
  The image is processed in 5 row-stripes (4 x 126 output rows + an
  8-row tail); a stripe tile [128, 8*512] holds rows 126s-1..126s+126
  one-row-per-partition. With adjacent rows on adjacent partitions the
  conv's three ROW taps collapse into one banded stationary matrix
  W_dj[p,c] = -k[p-c,dj]/6, so each sample needs only 3 matmuls per
  stripe - the column taps dj ride on shifted PSUM output windows
  (dj=1 full width with start=True, dj=0 into cols 1.., dj=2 into
  cols ..511), which also kills all column padding. x itself never
  passes through the PE: the host fold g = x + f/6 makes the single
  fused DVE blend out = g + psum complete the update. Matmuls issue
  dj-major across samples into 4 two-bank PSUM pair tiles (ILP across
  banks; one wide [126, 1024] blend per pair halves DVE overhead),
  and 16 zero matmuls warm the PE HAM clock gate (starts at ~half
  clock; K=128 activity opens it) while the first loads fly.
  The 24 banded weights are built host-side from kernelA and shipped
  as one [128, 24, 126] bf16 tensor, loaded first on the Scalar ring.
  Loads issue on Sync (x) / Scalar (wts, g) HWDGE rings, stores on
  the GpSimd SWDGE ring so a store waiting on compute never
  head-of-line-blocks a load. Output is stored bf16 [H, B, W]; host
  casts/untransposes to f32 [B,1,H,W]. All wire tensors are bf16
  (13.5MB/core total vs 25.2 in f32; rel err ~2.5e-3, gate 2e-2).
"""
import numpy as np
import concourse.bass as bass
import concourse.mybir as mybir
from concourse.tile import TileContext
from concourse.bass_utils import run_bass_kernel_spmd

F32 = mybir.dt.float32
BF16 = mybir.dt.bfloat16
NPBF16 = mybir.dt.np(BF16)
ALU = mybir.AluOpType

N_CORES = 8
BPC = 8          # samples per core
H = W = 512
SH = 126         # output rows per full stripe
NS = 5           # stripes (4 full + tail)
TAIL = H - 4 * SH  # 8

_MAX_WAITS = 1


def _fixup_sync_waits(nc):
    """This walrus build rejects >1-2 sem-waits per instruction; move the
    excess onto NOPs inserted just before, on the same engine (same program
    order, so semantics are unchanged)."""
    n_fix = 0
    for fn in nc.m.functions:
        for blk in fn.blocks:
            out, changed = [], False
            for inst in blk.instructions:
                si = inst.sync_info
                waits = list(si.on_wait or []) if si is not None else []
                if len(waits) > _MAX_WAITS:
                    changed = True
                    n_fix += 1
                    for i in range(0, len(waits) - _MAX_WAITS, _MAX_WAITS):
                        nop = mybir.InstNoOp(
                            name=f"I-waitfix-{nc.next_id()}", ins=[], outs=[])
                        nop.engine = inst.engine
                        nop.sync_info = mybir.SyncInfo(
                            on_wait=waits[i:i + _MAX_WAITS], on_update=[])
                        out.append(nop)
                    inst.sync_info = mybir.SyncInfo(
                        on_wait=waits[len(waits) - _MAX_WAITS:],
                        on_update=list(si.on_update or []))
                out.append(inst)
            if changed:
                blk.instructions = out
    return n_fix


def gen_kernel(n_samples=BPC):
    nc = bass.Bass(target_bir_lowering=False)
    # x is host-padded with a zero row on top and bottom ([H+2, B, W]) so
    # every stripe load covers a partition range starting at 0: a dst
    # partition range starting elsewhere (e.g. [1:128]) defeats the
    # DGE's per-engine descriptor split - all descriptors land on ONE
    # SDMA engine and the transfer serializes at ~27 GB/s.
    x = nc.dram_tensor("x", [H + 2, n_samples, W], BF16,
                       kind="ExternalInput")
    g = nc.dram_tensor("g", [H, n_samples, W], BF16, kind="ExternalInput")
    wts = nc.dram_tensor("wts", [128, 3 * n_samples + 3, SH], BF16,
                         kind="ExternalInput")
    out = nc.dram_tensor("out", [H, n_samples, W], BF16,
                         kind="ExternalOutput")

    BW = n_samples * W

    with TileContext(nc) as tc:
        with tc.tile_pool(name="const", bufs=1) as cpool, \
             tc.tile_pool(name="data", bufs=3) as dpool, \
             tc.tile_pool(name="psum", bufs=4, space="PSUM") as ppool:

            # wts rides first on the Scalar HWDGE queue: it must not delay
            # the first x stripe (Sync queue), and SWDGE (GpSimd) emits
            # descriptors ~8x slower. g only feeds blends, which trail the
            # first matmuls anyway.
            scr = cpool.tile([128, W], BF16)
            nc.gpsimd.memset(scr[:], 0.0)
            wt = cpool.tile([128, 3 * n_samples + 3, SH], BF16)
            half_slots = 3 * (n_samples // 2)
            nc.scalar.dma_start(out=wt[:, 0:half_slots, :],
                                in_=wts[:, 0:half_slots, :])
            nc.scalar.dma_start(out=wt[:, half_slots:, :],
                                in_=wts[:, half_slots:, :])

            # PE HAM warm-up: the clock gate starts at ~half rate and needs
            # ~4us of sustained activity to open. Burn zero matmuls on a
            # scratch tile while the first loads are in flight.
            wps = ppool.tile([128, 2 * W], F32, tag="ps", name="warm")
            NWARM = 24
            for i in range(NWARM):
                nc.tensor.matmul(wps[:, 0:W], scr[:, 0:128], scr[:],
                                 start=(i == 0), stop=(i == NWARM - 1))

            for s in (0, 1, 2, 3):
                kdim = 128
                cdim = SH

                xs = dpool.tile([128, BW], BF16, tag="xs")
                gs = dpool.tile([128, BW], BF16, tag="gs")
                ol = dpool.tile([128, BW], BF16, tag="ol")

                # stripe tile partition p holds image row SH*s + p - 1
                # (= padded-x row SH*s + p; rows -1 and H are host zeros).
                nc.sync.dma_start(
                    out=xs[0:kdim, :],
                    in_=x[SH * s:SH * s + kdim].rearrange(
                        "p b c -> p (b c)"))
                nc.scalar.dma_start(
                    out=gs[0:cdim, :],
                    in_=g[SH * s:SH * s + cdim].rearrange("p b c -> p (b c)"))

                # dj-major across samples: consecutive matmuls hit different
                # PSUM banks (all 8 in flight), so the engine's reorder
                # window can prefetch LDWEIGHTS and back-to-back matmuls
                # keep the PE array duty high enough that the HAM clock
                # gate stays open.
                pss = [ppool.tile([128, 2 * W], F32, tag="ps",
                                  name=f"ps{s}_{bp}")
                       for bp in range(n_samples // 2)]
                for dji, dj in enumerate((1, 0, 2)):
                    for b in range(n_samples):
                        o = b * W
                        po = (b % 2) * W
                        ps = pss[b // 2][:, po:po + W]
                        if dj == 1:   # center tap: full width, clears
                            nc.tensor.matmul(
                                ps[0:cdim, :], wt[0:kdim, 3 * b + 1, 0:cdim],
                                xs[0:kdim, o:o + W], start=True, stop=False)
                        elif dj == 0:  # out col j taps x col j-1
                            nc.tensor.matmul(
                                ps[0:cdim, 1:W], wt[0:kdim, 3 * b, 0:cdim],
                                xs[0:kdim, o:o + W - 1],
                                start=False, stop=False)
                        else:          # dj=2: out col j taps x col j+1
                            nc.tensor.matmul(
                                ps[0:cdim, 0:W - 1],
                                wt[0:kdim, 3 * b + 2, 0:cdim],
                                xs[0:kdim, o + 1:o + W],
                                start=False, stop=True)

                for bp in range(n_samples // 2):
                    o = 2 * bp * W
                    # blend: out = (x + f/6) + psum, two samples per DVE
                    # op (the pair PSUM tile spans 2 banks) to halve the
                    # per-op fixed overhead; x folded into g host-side.
                    nc.vector.scalar_tensor_tensor(
                        out=ol[0:cdim, o:o + 2 * W],
                        in0=gs[0:cdim, o:o + 2 * W],
                        scalar=1.0, in1=pss[bp][0:cdim, :],
                        op0=ALU.mult, op1=ALU.add)

                if s == 3:
                    # split the final full-stripe store so its 1MB drain
                    # starts as soon as the first two pairs are blended
                    for hh in range(2):
                        nc.gpsimd.dma_start(
                            out=out[SH * s:SH * s + cdim,
                                    4 * hh:4 * hh + 4, :].rearrange(
                                "p b c -> p (b c)"),
                            in_=ol[0:cdim, 4 * hh * W:(4 * hh + 4) * W])
                else:
                    nc.gpsimd.dma_start(
                        out=out[SH * s:SH * s + cdim].rearrange(
                            "p b c -> p (b c)"),
                        in_=ol[0:cdim, :])

            # Tail (last 8 rows of every sample) as ONE block-diagonal
            # stripe: partition 10b+r holds padded row 504+r of sample b,
            # so W_tail[80, 64] (8 diagonal [10, 8] bands, one per sample)
            # computes all samples' tail rows in 3 matmuls + 1 blend
            # instead of 24 matmuls + 4 blends.
            KT = 10 * n_samples   # 80 input partitions
            CT = TAIL * n_samples  # 64 output partitions
            xs = dpool.tile([128, BW], BF16, tag="xs")
            gs = dpool.tile([128, BW], BF16, tag="gs")
            ol = dpool.tile([128, BW], BF16, tag="ol")
            xap = x[4 * SH:4 * SH + 10]
            nc.sync.dma_start(
                out=xs[0:KT, 0:W],
                in_=bass.AP(xap.tensor, xap.offset,
                            [[W, n_samples], [BW, 10], [1, W]]))
            gap = g[4 * SH:H]
            nc.scalar.dma_start(
                out=gs[0:CT, 0:W],
                in_=bass.AP(gap.tensor, gap.offset,
                            [[W, n_samples], [BW, TAIL], [1, W]]))
            ps = ppool.tile([128, 2 * W], F32, tag="ps", name="ps_tail")
            TS = 3 * n_samples  # first tail weight slot
            nc.tensor.matmul(ps[0:CT, 0:W], wt[0:KT, TS + 1, 0:CT],
                             xs[0:KT, 0:W], start=True, stop=False)
            nc.tensor.matmul(ps[0:CT, 1:W], wt[0:KT, TS, 0:CT],
                             xs[0:KT, 0:W - 1], start=False, stop=False)
            nc.tensor.matmul(ps[0:CT, 0:W - 1], wt[0:KT, TS + 2, 0:CT],
                             xs[0:KT, 1:W], start=False, stop=True)
            nc.vector.scalar_tensor_tensor(
                out=ol[0:CT, 0:W], in0=gs[0:CT, 0:W], scalar=1.0,
                in1=ps[0:CT, 0:W], op0=ALU.mult, op1=ALU.add)
            oap = out[4 * SH:H]
            nc.gpsimd.dma_start(
                out=bass.AP(oap.tensor, oap.offset,
                            [[W, n_samples], [BW, TAIL], [1, W]]),
                in_=ol[0:CT, 0:W])
    return nc


def _make_wts(kA):
    """[128, 27, 126] bf16: slot 3b+dj holds the banded conv weight
    W[p, c] = -kA[b, 0, p-c, dj]/6 (p-c in 0..2); slots 24..26 hold the
    block-diagonal tail weights W[10b+r+di, 8b+r] = -kA[b, 0, di, dj]/6
    that compute every sample's last 8 rows in one matmul per dj."""
    w = np.zeros((128, 3 * BPC + 3, SH), np.float32)
    c = np.arange(SH)
    r = np.arange(TAIL)
    for b in range(BPC):
        for dj in range(3):
            for di in range(3):
                w[c + di, 3 * b + dj, c] = -kA[b, 0, di, dj] / 6.0
                w[10 * b + r + di, 3 * BPC + dj, TAIL * b + r] = \
                    -kA[b, 0, di, dj] / 6.0
    return w.astype(NPBF16)


def _make_in_maps(x, f, kernelA):
    in_maps = []
    for cid in range(N_CORES):
        s = slice(cid * BPC, (cid + 1) * BPC)
        # [B, 1, H, W] -> [H+2, B, W]: one row across all samples is 8KB;
        # zero halo rows at top/bottom keep all loads partition-0-based.
        xt = np.zeros((H + 2, BPC, W), dtype=NPBF16)
        xt[1:H + 1] = x[s, 0].transpose(1, 0, 2).astype(NPBF16)
        # g = x + f/6 folded host-side: the blend adds it in one DVE op,
        # so no identity matmul is needed to bring x into PSUM.
        gt = np.ascontiguousarray(
            (x[s, 0] + f[s, 0] * (1.0 / 6.0)).transpose(1, 0, 2)
        ).astype(NPBF16)
        in_maps.append({"x": xt, "g": gt, "wts": _make_wts(kernelA[s])})
    return in_maps


def run_sharded(x, f, kernelA, trace=False):
    """Compile+run on 8 cores; returns (full output, BassKernelResults)."""
    x = np.asarray(x, dtype=np.float32)
    f = np.asarray(f, dtype=np.float32)
    kernelA = np.asarray(kernelA, dtype=np.float32)
    nc = gen_kernel()
    _fixup_sync_waits(nc)
    res = run_bass_kernel_spmd(nc, _make_in_maps(x, f, kernelA),
                               core_ids=list(range(N_CORES)), trace=trace)
    out = np.concatenate(
        [res.results[c]["out"].astype(np.float32)
         .transpose(1, 0, 2).reshape(BPC, 1, H, W)
         for c in range(N_CORES)], axis=0)
    return out, res


def kernel(x, f, kernelA):
    out, _ = run_sharded(x, f, kernelA, trace=False)
    return out
